# revision 7
# baseline (speedup 1.0000x reference)
"""Biaffine attention kernel for Trainium2, data-parallel over 8 NeuronCores.

Math (per batch b):
    xp = Wf @ x[b] + bf          (128, L)
    yp = Wa @ y[b] + ba          (128, L)
    scores = xp @ yp.T           (128, 128)   contraction over L
    attn = softmax(scores, -1) / sqrt(L)
    out[b] = attn @ (xp + yp)    (128, L)

Distribution: batch dim (32) sharded 4-per-core across 8 cores; weights
replicated. No collectives.

Per-core dataflow (software-pipelined across batches):
  - x/y streamed HBM->SBUF in 2 MiB tiles, cast fp32->fp16 in the DMA
    datapath (SWDGE), 2-batch prefetch depth.
  - projections as fp16 matmuls (N=512), weights pre-transposed and
    pre-cast to fp16 on the host; PSUM evacuated by ScalarE with fused
    per-partition bias, activations kept fp16 in SBUF.
  - xp16/yp16 transposed 128x128 via TensorE transpose-mode; scores
    accumulate over 64 chunks into one PSUM bank.
  - softmax rowwise; 1/sqrt(L) folded into the normalizer.
  - out = attnT.T @ xp16 + attnT.T @ yp16 in PSUM, streamed back fp32
    in 1 MiB stores.
  - emission order interleaves phase 1 of batch b+1 into the softmax
    window of batch b so TensorE/ACT never drain between batches.
"""

import numpy as np

P = 128
L = 8192
B = 32
NCORES = 8
BPC = B // NCORES  # batches per core
SQRT_L = float(np.sqrt(float(L)))

CHUNK = 512  # projection / out matmul free dim
TCH = 128  # transpose chunk
TGRP = 8  # transposes per PSUM bank evacuation
IN_TILE = 4096  # HBM->SBUF dma tile (2 MiB fp32 read, fp16 SBUF write)
IN_BUFS = 4  # input tile slots per tensor (2-batch lookahead)
OUT_TILE = 2048  # SBUF->HBM store tile (1 MiB fp32)


def _patch_tail_drain(tile, mybir, ScopedClock):
    """This container's walrus rejects >1 sync wait on the kernel-tail Drain
    (setupSyncWait: 'Too many sync wait commands'). Spread the tail-drain
    waits across a chain of drains, one wait each."""
    if getattr(tile.TileContext, "_drain_split_patched", False):
        return

    def _split_drain_and_barrier(self, tick_clock, wait_clock):
        nc = self.nc
        drain_inst = nc.sync.drain()
        wait_clock.add_sem_waits(
            drain_inst.ins, ScopedClock({None: tick_clock.global_clock})
        )
        si = drain_inst.ins.sync_info
        if si is not None and si.on_wait is not None and len(si.on_wait) > 1:
            waits = list(si.on_wait)
            si.on_wait = waits[:1]
            for w in waits[1:]:
                extra = nc.sync.drain()
                esi = extra.ins.sync_info
                if esi is None:
                    extra.ins.sync_info = mybir.SyncInfo(on_wait=[w], on_update=[])
                else:
                    ow = list(esi.on_wait) if esi.on_wait else []
                    ow.append(w)
                    esi.on_wait = ow
        nc.all_engine_barrier()
        assert self.sems is not None
        popped = nc._tile_sem_poison_stack.pop()
        assert popped is self._sem_poison
        nc.clear_and_free_semaphores(list(self.sems.allocated().values()))
        nc.all_engine_barrier()

    tile.TileContext._drain_and_barrier = _split_drain_and_barrier
    tile.TileContext._drain_split_patched = True


def _split_excess_waits(nc, mybir, max_waits=1):
    """Walrus in this container rejects instructions carrying more than a
    couple of sync waits ('Too many sync wait commands'). Hoist excess waits
    onto dedicated same-engine NoOps inserted just before the instruction."""
    ctr = 0
    for blk in nc.m.functions[0].blocks:
        new_insts = []
        for inst in blk.instructions:
            si = inst.sync_info
            if si is not None and si.on_wait and len(si.on_wait) > max_waits:
                waits = list(si.on_wait)
                excess, keep = waits[:-max_waits], waits[-max_waits:]
                si.on_wait = keep
                for i in range(0, len(excess), max_waits):
                    ctr += 1
                    nop = mybir.InstNoOp(
                        name=f"I-waitsplit-{ctr}",
                        sync_info=mybir.SyncInfo(
                            on_wait=excess[i : i + max_waits], on_update=[]
                        ),
                        bass_nofuse=True,
                        engine=inst.engine,
                    )
                    nc.register_instruction(nop)
                    new_insts.append(nop)
            new_insts.append(inst)
        blk.instructions = new_insts


def build_nc(bpc=BPC, seq=L):
    import concourse.bass as bass
    import concourse.mybir as mybir
    import concourse.tile as tile
    from concourse.masks import make_identity
    from concourse.vector_clock import ScopedClock

    _patch_tail_drain(tile, mybir, ScopedClock)

    f32 = mybir.dt.float32
    f16 = mybir.dt.float16
    AF = mybir.ActivationFunctionType
    ALU = mybir.AluOpType
    AX = mybir.AxisListType

    sqrt_l = float(np.sqrt(float(seq)))
    in_tile = min(IN_TILE, seq)
    ntr = seq // TCH  # number of 128-col transpose chunks
    tgrp = min(TGRP, ntr)  # transposes per PSUM bank
    nin = seq // in_tile  # dma tiles per batch
    cpin = in_tile // CHUNK  # matmul chunks per dma tile
    out_tile = min(OUT_TILE, seq)
    nout = seq // out_tile
    cpo = out_tile // CHUNK

    nc = bass.Bass("TRN2", target_bir_lowering=False, debug=False)
    x_d = nc.dram_tensor("x", [bpc, P, seq], f32, kind="ExternalInput").ap()
    y_d = nc.dram_tensor("y", [bpc, P, seq], f32, kind="ExternalInput").ap()
    # weights arrive pre-transposed ([in, out]) and pre-cast to fp16 from
    # the host; biases arrive as [P, 1] fp32.
    wft_d = nc.dram_tensor("wft", [P, P], f16, kind="ExternalInput").ap()
    bf_d = nc.dram_tensor("bf", [P, 1], f32, kind="ExternalInput").ap()
    wat_d = nc.dram_tensor("wat", [P, P], f16, kind="ExternalInput").ap()
    ba_d = nc.dram_tensor("ba", [P, 1], f32, kind="ExternalInput").ap()
    out_d = nc.dram_tensor("out", [bpc, P, seq], f32, kind="ExternalOutput").ap()

    with tile.TileContext(nc) as tc:
        with (
            tc.tile_pool(name="consts", bufs=1) as consts,
            tc.tile_pool(name="xin", bufs=IN_BUFS) as xin_pool,
            tc.tile_pool(name="acts", bufs=2) as acts_pool,
            tc.tile_pool(name="trs", bufs=1) as tr_pool,
            tc.tile_pool(name="sm", bufs=2) as sm_pool,
            tc.tile_pool(name="outs", bufs=2) as out_pool,
            tc.tile_pool(name="pproj", bufs=3, space="PSUM") as psum_proj,
            tc.tile_pool(name="ptr", bufs=2, space="PSUM") as psum_tr,
            tc.tile_pool(name="psc", bufs=1, space="PSUM") as psum_sc,
            tc.tile_pool(name="pout", bufs=2, space="PSUM") as psum_out,
        ):
            pending = {}  # (b, h) -> (x_tile, y_tile)

            def emit_loads(b):
                if b >= bpc:
                    return
                for h in range(nin):
                    x_t = xin_pool.tile([P, in_tile], f16, tag="x_t", name="x_t")
                    y_t = xin_pool.tile([P, in_tile], f16, tag="y_t", name="y_t")
                    hs = slice(h * in_tile, (h + 1) * in_tile)
                    nc.gpsimd.dma_start(x_t, x_d[b, :, hs])
                    nc.gpsimd.dma_start(y_t, y_d[b, :, hs])
                    pending[(b, h)] = (x_t, y_t)

            # 2-batch deep prefetch before anything else (program order
            # drives scheduler priority).
            emit_loads(0)
            emit_loads(1)

            # ---- constants ----
            wfT = consts.tile([P, P], f16)
            nc.sync.dma_start(wfT, wft_d)
            waT = consts.tile([P, P], f16)
            nc.sync.dma_start(waT, wat_d)
            bias_f = consts.tile([P, 1], f32)
            nc.sync.dma_start(bias_f, bf_d)
            bias_a = consts.tile([P, 1], f32)
            nc.sync.dma_start(bias_a, ba_d)
            ids = consts.tile([P, P], f16)
            make_identity(nc, ids)

            acts = {}  # b -> (xp16, yp16)

            def phase1(b):
                """Stream in + project + bias; also emits loads for b+2."""
                if b >= bpc:
                    return
                xp16 = acts_pool.tile([P, seq], f16, tag="xp16", name="xp16")
                yp16 = acts_pool.tile([P, seq], f16, tag="yp16", name="yp16")
                acts[b] = (xp16, yp16)
                for h in range(nin):
                    x_t, y_t = pending.pop((b, h))
                    for cc in range(cpin):
                        c0 = h * in_tile + cc * CHUNK
                        cs_in = slice(cc * CHUNK, (cc + 1) * CHUNK)
                        cs = slice(c0, c0 + CHUNK)
                        px = psum_proj.tile([P, CHUNK], f32, tag="pp", name="px")
                        nc.tensor.matmul(
                            px, wfT[:], x_t[:, cs_in], start=True, stop=True
                        )
                        # split evacuation across ACT (x) and DVE (y) so the
                        # projection phase isn't single-server bound
                        nc.scalar.activation(
                            out=xp16[:, cs], in_=px, func=AF.Identity, bias=bias_f
                        )
                        py = psum_proj.tile([P, CHUNK], f32, tag="pp", name="py")
                        nc.tensor.matmul(
                            py, waT[:], y_t[:, cs_in], start=True, stop=True
                        )
                        nc.vector.tensor_scalar_add(yp16[:, cs], py, bias_a)
                    # slots for (b, h) free after the matmuls above; queue
                    # the same-index loads of batch b+2 behind them.
                    if b + 2 < bpc:
                        bb = b + 2
                        x_t2 = xin_pool.tile([P, in_tile], f16, tag="x_t", name="x_t")
                        y_t2 = xin_pool.tile([P, in_tile], f16, tag="y_t", name="y_t")
                        hs = slice(h * in_tile, (h + 1) * in_tile)
                        nc.gpsimd.dma_start(x_t2, x_d[bb, :, hs])
                        nc.gpsimd.dma_start(y_t2, y_d[bb, :, hs])
                        pending[(bb, h)] = (x_t2, y_t2)

            trs = {}  # b -> (xpT, ypT)

            def phase2(b):
                """Transpose activations 128x128 via TensorE."""
                if b >= bpc:
                    return
                xp16, yp16 = acts[b]
                xpT = tr_pool.tile([P, seq], f16, tag="xpT", name="xpT")
                ypT = tr_pool.tile([P, seq], f16, tag="ypT", name="ypT")
                trs[b] = (xpT, ypT)
                for g in range(ntr // tgrp):
                    ptx = psum_tr.tile([P, tgrp * TCH], f16, tag="pt", name="ptx")
                    pty = psum_tr.tile([P, tgrp * TCH], f16, tag="pt", name="pty")
                    for t in range(tgrp):
                        c = g * tgrp + t
                        ts_ = slice(t * TCH, (t + 1) * TCH)
                        cs = slice(c * TCH, (c + 1) * TCH)
                        nc.tensor.transpose(ptx[:, ts_], xp16[:, cs], ids)
                        nc.tensor.transpose(pty[:, ts_], yp16[:, cs], ids)
                    gs = slice(g * tgrp * TCH, (g + 1) * tgrp * TCH)
                    nc.vector.tensor_copy(out=xpT[:, gs], in_=ptx)
                    nc.vector.tensor_copy(out=ypT[:, gs], in_=pty)

            def phase3(b):
                """Scores: accumulate xpT.T @ ypT over seq chunks."""
                xpT, ypT = trs[b]
                ps = psum_sc.tile([P, P], f32, tag="ps", name="ps")
                for c in range(ntr):
                    cs = slice(c * TCH, (c + 1) * TCH)
                    nc.tensor.matmul(
                        ps,
                        xpT[:, cs],
                        ypT[:, cs],
                        start=(c == 0),
                        stop=(c == ntr - 1),
                    )
                return ps

            def softmax_head(b, ps):
                negmx = sm_pool.tile([P, 1], f32, tag="negmx", name="negmx")
                nc.vector.tensor_reduce(
                    out=negmx, in_=ps, axis=AX.X, op=ALU.max, negate=True
                )
                e = sm_pool.tile([P, P], f32, tag="e", name="e")
                se = sm_pool.tile([P, 1], f32, tag="se", name="se")
                nc.scalar.activation(
                    out=e, in_=ps, func=AF.Exp, bias=negmx, scale=1.0, accum_out=se
                )
                return e, se

            def softmax_tail(b, e, se):
                sse = sm_pool.tile([P, 1], f32, tag="sse", name="sse")
                nc.vector.tensor_scalar_mul(sse, se, sqrt_l)
                rcp = sm_pool.tile([P, 1], f32, tag="rcp", name="rcp")
                nc.vector.reciprocal(rcp, sse)
                attn = sm_pool.tile([P, P], f16, tag="attn", name="attn")
                nc.vector.tensor_scalar_mul(attn, e, rcp)
                pat = psum_tr.tile([P, tgrp * TCH], f16, tag="pt", name="pat")
                nc.tensor.transpose(pat[:, :P], attn, ids)
                attnT = sm_pool.tile([P, P], f16, tag="attnT", name="attnT")
                nc.vector.tensor_copy(out=attnT, in_=pat[:, :P])
                return attnT

            def phase5(b, attnT):
                """out = attnT.T @ (xp + yp), stream back fp32."""
                xp16, yp16 = acts[b]
                for h in range(nout):
                    ot = out_pool.tile([P, out_tile], f32, tag="ot", name="ot")
                    for cc in range(cpo):
                        c0 = h * out_tile + cc * CHUNK
                        cs = slice(c0, c0 + CHUNK)
                        po = psum_out.tile([P, CHUNK], f32, tag="po", name="po")
                        nc.tensor.matmul(
                            po, attnT[:], xp16[:, cs], start=True, stop=False
                        )
                        nc.tensor.matmul(
                            po, attnT[:], yp16[:, cs], start=False, stop=True
                        )
                        nc.any.tensor_copy(
                            out=ot[:, cc * CHUNK : (cc + 1) * CHUNK], in_=po
                        )
                    hs = slice(h * out_tile, (h + 1) * out_tile)
                    # stores issue from the ACT HWDGE ring so they don't
                    # share a ring with anything hot
                    nc.scalar.dma_start(out_d[b, :, hs], ot)

            # ---- emission: phase order per batch; loads prefetched two
            # batches ahead keep DMA dense across phase boundaries ----
            for b in range(bpc):
                phase1(b)
                phase2(b)
                ps = phase3(b)
                e, se = softmax_head(b, ps)
                attnT = softmax_tail(b, e, se)
                phase5(b, attnT)

    _split_excess_waits(nc, mybir, max_waits=1)
    return nc


_nc_cache = {}


def _get_nc():
    key = (BPC, L)
    if key not in _nc_cache:
        _nc_cache[key] = build_nc(BPC, L)
    return _nc_cache[key]


def _in_maps(x, y, Wf, bf, Wa, ba):
    wft = np.ascontiguousarray(np.asarray(Wf, dtype=np.float32).T).astype(np.float16)
    wat = np.ascontiguousarray(np.asarray(Wa, dtype=np.float32).T).astype(np.float16)
    bf2 = np.ascontiguousarray(np.asarray(bf, dtype=np.float32).reshape(P, 1))
    ba2 = np.ascontiguousarray(np.asarray(ba, dtype=np.float32).reshape(P, 1))
    maps = []
    for c in range(NCORES):
        sl = slice(c * BPC, (c + 1) * BPC)
        maps.append(
            {
                "x": np.ascontiguousarray(x[sl]),
                "y": np.ascontiguousarray(y[sl]),
                "wft": wft,
                "bf": bf2,
                "wat": wat,
                "ba": ba2,
            }
        )
    return maps


def kernel(x, y, Wf, bf, Wa, ba):
    from concourse.bass_utils import run_bass_kernel_spmd

    x = np.asarray(x, dtype=np.float32)
    y = np.asarray(y, dtype=np.float32)
    nc = _get_nc()
    res = run_bass_kernel_spmd(
        nc, _in_maps(x, y, Wf, bf, Wa, ba), core_ids=list(range(NCORES))
    )
    out = np.concatenate([r["out"] for r in res.results], axis=0)
    return np.ascontiguousarray(out.astype(np.float32))


if __name__ == "__main__":
    rng = np.random.default_rng(0)
    inputs = {
        "x": rng.standard_normal((B, P, L), dtype=np.float32),
        "y": rng.standard_normal((B, P, L), dtype=np.float32),
        "Wf": (rng.standard_normal((P, P)) / np.sqrt(P)).astype(np.float32),
        "bf": (rng.standard_normal(P) * 0.02).astype(np.float32),
        "Wa": (rng.standard_normal((P, P)) / np.sqrt(P)).astype(np.float32),
        "ba": (rng.standard_normal(P) * 0.02).astype(np.float32),
    }
    o = kernel(**inputs)
    print(o.shape, o.dtype)


# revision 11
# speedup vs baseline: 1.2087x; 1.2087x over previous
"""Biaffine attention kernel for Trainium2, data-parallel over 8 NeuronCores.

Math (per batch b):
    xp = Wf @ x[b] + bf          (128, L)
    yp = Wa @ y[b] + ba          (128, L)
    scores = xp @ yp.T           (128, 128)   contraction over L
    attn = softmax(scores, -1) / sqrt(L)
    out[b] = attn @ (xp + yp)    (128, L)

Distribution: batch dim (32) sharded 4-per-core across 8 cores; weights
replicated. No collectives.

Per-core dataflow (software-pipelined across batches):
  - x/y streamed HBM->SBUF in 2 MiB tiles, cast fp32->fp16 in the DMA
    datapath (SWDGE), 2-batch prefetch depth.
  - projections as fp16 matmuls (N=512), weights pre-transposed and
    pre-cast to fp16 on the host; PSUM evacuated by ScalarE with fused
    per-partition bias, activations kept fp16 in SBUF.
  - xp16/yp16 transposed 128x128 via TensorE transpose-mode; scores
    accumulate over 64 chunks into one PSUM bank.
  - softmax rowwise; 1/sqrt(L) folded into the normalizer.
  - out = attnT.T @ xp16 + attnT.T @ yp16 in PSUM, streamed back fp32
    in 1 MiB stores.
  - emission order interleaves phase 1 of batch b+1 into the softmax
    window of batch b so TensorE/ACT never drain between batches.
"""

import numpy as np

P = 128
L = 8192
B = 32
NCORES = 8
BPC = B // NCORES  # batches per core
SQRT_L = float(np.sqrt(float(L)))

CHUNK = 512  # projection / out matmul free dim
TCH = 128  # transpose chunk
TGRP = 8  # transposes per PSUM bank evacuation
IN_TILE = 4096  # HBM->SBUF dma tile (2 MiB fp32 read, fp16 SBUF write)
IN_BUFS = 4  # input tile slots per tensor (2-batch lookahead)
OUT_TILE = 2048  # SBUF->HBM store tile (1 MiB fp32)


def _patch_tail_drain(tile, mybir, ScopedClock):
    """This container's walrus rejects >1 sync wait on the kernel-tail Drain
    (setupSyncWait: 'Too many sync wait commands'). Spread the tail-drain
    waits across a chain of drains, one wait each."""
    if getattr(tile.TileContext, "_drain_split_patched", False):
        return

    def _split_drain_and_barrier(self, tick_clock, wait_clock):
        nc = self.nc
        drain_inst = nc.sync.drain()
        wait_clock.add_sem_waits(
            drain_inst.ins, ScopedClock({None: tick_clock.global_clock})
        )
        si = drain_inst.ins.sync_info
        if si is not None and si.on_wait is not None and len(si.on_wait) > 1:
            waits = list(si.on_wait)
            si.on_wait = waits[:1]
            for w in waits[1:]:
                extra = nc.sync.drain()
                esi = extra.ins.sync_info
                if esi is None:
                    extra.ins.sync_info = mybir.SyncInfo(on_wait=[w], on_update=[])
                else:
                    ow = list(esi.on_wait) if esi.on_wait else []
                    ow.append(w)
                    esi.on_wait = ow
        nc.all_engine_barrier()
        assert self.sems is not None
        popped = nc._tile_sem_poison_stack.pop()
        assert popped is self._sem_poison
        nc.clear_and_free_semaphores(list(self.sems.allocated().values()))
        nc.all_engine_barrier()

    tile.TileContext._drain_and_barrier = _split_drain_and_barrier
    tile.TileContext._drain_split_patched = True


def _split_excess_waits(nc, mybir, max_waits=1):
    """Walrus in this container rejects instructions carrying more than a
    couple of sync waits ('Too many sync wait commands'). Hoist excess waits
    onto dedicated same-engine NoOps inserted just before the instruction."""
    ctr = 0
    for blk in nc.m.functions[0].blocks:
        new_insts = []
        for inst in blk.instructions:
            si = inst.sync_info
            if si is not None and si.on_wait and len(si.on_wait) > max_waits:
                waits = list(si.on_wait)
                excess, keep = waits[:-max_waits], waits[-max_waits:]
                si.on_wait = keep
                for i in range(0, len(excess), max_waits):
                    ctr += 1
                    nop = mybir.InstNoOp(
                        name=f"I-waitsplit-{ctr}",
                        sync_info=mybir.SyncInfo(
                            on_wait=excess[i : i + max_waits], on_update=[]
                        ),
                        bass_nofuse=True,
                        engine=inst.engine,
                    )
                    nc.register_instruction(nop)
                    new_insts.append(nop)
            new_insts.append(inst)
        blk.instructions = new_insts


def build_nc(bpc=BPC, seq=L):
    import concourse.bass as bass
    import concourse.mybir as mybir
    import concourse.tile as tile
    from concourse.masks import make_identity
    from concourse.vector_clock import ScopedClock

    _patch_tail_drain(tile, mybir, ScopedClock)

    f32 = mybir.dt.float32
    f16 = mybir.dt.float16
    AF = mybir.ActivationFunctionType
    ALU = mybir.AluOpType
    AX = mybir.AxisListType

    sqrt_l = float(np.sqrt(float(seq)))
    in_tile = min(IN_TILE, seq)
    ntr = seq // TCH  # number of 128-col transpose chunks
    tgrp = min(TGRP, ntr)  # transposes per PSUM bank
    nin = seq // in_tile  # dma tiles per batch
    cpin = in_tile // CHUNK  # matmul chunks per dma tile
    out_tile = min(OUT_TILE, seq)
    nout = seq // out_tile
    cpo = out_tile // CHUNK

    nc = bass.Bass("TRN2", target_bir_lowering=False, debug=False)
    x_d = nc.dram_tensor("x", [bpc, P, seq], f32, kind="ExternalInput").ap()
    y_d = nc.dram_tensor("y", [bpc, P, seq], f32, kind="ExternalInput").ap()
    # weights arrive pre-transposed ([in, out]) and pre-cast to fp16 from
    # the host; biases arrive as [P, 1] fp32.
    wft_d = nc.dram_tensor("wft", [P, P], f16, kind="ExternalInput").ap()
    bf_d = nc.dram_tensor("bf", [P, 1], f32, kind="ExternalInput").ap()
    wat_d = nc.dram_tensor("wat", [P, P], f16, kind="ExternalInput").ap()
    ba_d = nc.dram_tensor("ba", [P, 1], f32, kind="ExternalInput").ap()
    out_d = nc.dram_tensor("out", [bpc, P, seq], f32, kind="ExternalOutput").ap()

    with tile.TileContext(nc) as tc:
        with (
            tc.tile_pool(name="consts", bufs=1) as consts,
            tc.tile_pool(name="xin", bufs=IN_BUFS) as xin_pool,
            tc.tile_pool(name="acts", bufs=2) as acts_pool,
            tc.tile_pool(name="trs", bufs=1) as tr_pool,
            tc.tile_pool(name="sm", bufs=2) as sm_pool,
            tc.tile_pool(name="outs", bufs=2) as out_pool,
            tc.tile_pool(name="pproj", bufs=2, space="PSUM") as psum_proj,
            tc.tile_pool(name="ptr", bufs=3, space="PSUM") as psum_tr,
            tc.tile_pool(name="psc", bufs=1, space="PSUM") as psum_sc,
            tc.tile_pool(name="pout", bufs=2, space="PSUM") as psum_out,
        ):
            pending = {}  # (b, h) -> (x_tile, y_tile)

            def emit_loads(b):
                if b >= bpc:
                    return
                for h in range(nin):
                    x_t = xin_pool.tile([P, in_tile], f16, tag="x_t", name="x_t")
                    y_t = xin_pool.tile([P, in_tile], f16, tag="y_t", name="y_t")
                    hs = slice(h * in_tile, (h + 1) * in_tile)
                    nc.gpsimd.dma_start(x_t, x_d[b, :, hs])
                    nc.gpsimd.dma_start(y_t, y_d[b, :, hs])
                    pending[(b, h)] = (x_t, y_t)

            # 2-batch deep prefetch before anything else (program order
            # drives scheduler priority).
            emit_loads(0)
            emit_loads(1)

            # ---- constants ----
            wfT = consts.tile([P, P], f16)
            nc.sync.dma_start(wfT, wft_d)
            waT = consts.tile([P, P], f16)
            nc.sync.dma_start(waT, wat_d)
            bias_f = consts.tile([P, 1], f32)
            nc.sync.dma_start(bias_f, bf_d)
            bias_a = consts.tile([P, 1], f32)
            nc.sync.dma_start(bias_a, ba_d)
            ids = consts.tile([P, P], f16)
            make_identity(nc, ids)

            acts = {}  # b -> (xp16, yp16)

            def p1_chunks(b):
                """Generator: emits one projection chunk (px+py matmuls and
                ACT evacuations) per next() call; emits loads for b+2 as
                input slots free."""
                if b >= bpc:
                    return
                xp16 = acts_pool.tile([P, seq], f16, tag="xp16", name="xp16")
                yp16 = acts_pool.tile([P, seq], f16, tag="yp16", name="yp16")
                acts[b] = (xp16, yp16)
                for h in range(nin):
                    x_t, y_t = pending.pop((b, h))
                    for cc in range(cpin):
                        c0 = h * in_tile + cc * CHUNK
                        cs_in = slice(cc * CHUNK, (cc + 1) * CHUNK)
                        cs = slice(c0, c0 + CHUNK)
                        px = psum_proj.tile([P, CHUNK], f32, tag="pp", name="px")
                        nc.tensor.matmul(
                            px, wfT[:], x_t[:, cs_in], start=True, stop=True
                        )
                        nc.scalar.activation(
                            out=xp16[:, cs], in_=px, func=AF.Identity, bias=bias_f
                        )
                        py = psum_proj.tile([P, CHUNK], f32, tag="pp", name="py")
                        nc.tensor.matmul(
                            py, waT[:], y_t[:, cs_in], start=True, stop=True
                        )
                        nc.scalar.activation(
                            out=yp16[:, cs], in_=py, func=AF.Identity, bias=bias_a
                        )
                        yield
                    # slots for (b, h) free after the matmuls above; queue
                    # the same-index loads of batch b+2 behind them.
                    if b + 2 < bpc:
                        bb = b + 2
                        x_t2 = xin_pool.tile([P, in_tile], f16, tag="x_t", name="x_t")
                        y_t2 = xin_pool.tile([P, in_tile], f16, tag="y_t", name="y_t")
                        hs = slice(h * in_tile, (h + 1) * in_tile)
                        nc.gpsimd.dma_start(x_t2, x_d[bb, :, hs])
                        nc.gpsimd.dma_start(y_t2, y_d[bb, :, hs])
                        pending[(bb, h)] = (x_t2, y_t2)

            def phase23(b):
                """Transpose activations and fold the just-transposed columns
                straight into the scores accumulation, so TensorE does scores
                matmuls while DVE evacuates the next transpose group."""
                xp16, yp16 = acts[b]
                xpT = tr_pool.tile([P, seq], f16, tag="xpT", name="xpT")
                ypT = tr_pool.tile([P, seq], f16, tag="ypT", name="ypT")
                ps = psum_sc.tile([P, P], f32, tag="ps", name="ps")
                ngr = ntr // tgrp

                def scores_group(g):
                    for t in range(tgrp):
                        c = g * tgrp + t
                        cs = slice(c * TCH, (c + 1) * TCH)
                        nc.tensor.matmul(
                            ps,
                            xpT[:, cs],
                            ypT[:, cs],
                            start=(c == 0),
                            stop=(c == ntr - 1),
                        )

                for g in range(ngr):
                    ptx = psum_tr.tile([P, tgrp * TCH], f16, tag="pt", name="ptx")
                    pty = psum_tr.tile([P, tgrp * TCH], f16, tag="pt", name="pty")
                    for t in range(tgrp):
                        c = g * tgrp + t
                        ts_ = slice(t * TCH, (t + 1) * TCH)
                        cs = slice(c * TCH, (c + 1) * TCH)
                        nc.tensor.transpose(ptx[:, ts_], xp16[:, cs], ids)
                        nc.tensor.transpose(pty[:, ts_], yp16[:, cs], ids)
                    gs = slice(g * tgrp * TCH, (g + 1) * tgrp * TCH)
                    nc.vector.tensor_copy(out=xpT[:, gs], in_=ptx)
                    nc.vector.tensor_copy(out=ypT[:, gs], in_=pty)
                    # scores lag the transposes by one group: TensorE
                    # accumulates group g-1 while DVE evacuates group g
                    if g >= 1:
                        scores_group(g - 1)
                scores_group(ngr - 1)
                return ps

            def softmax_head(b, ps):
                negmx = sm_pool.tile([P, 1], f32, tag="negmx", name="negmx")
                nc.vector.tensor_reduce(
                    out=negmx, in_=ps, axis=AX.X, op=ALU.max, negate=True
                )
                e = sm_pool.tile([P, P], f32, tag="e", name="e")
                se = sm_pool.tile([P, 1], f32, tag="se", name="se")
                nc.scalar.activation(
                    out=e, in_=ps, func=AF.Exp, bias=negmx, scale=1.0, accum_out=se
                )
                return e, se

            def softmax_tail(b, e, se):
                sse = sm_pool.tile([P, 1], f32, tag="sse", name="sse")
                nc.vector.tensor_scalar_mul(sse, se, sqrt_l)
                rcp = sm_pool.tile([P, 1], f32, tag="rcp", name="rcp")
                nc.vector.reciprocal(rcp, sse)
                attn = sm_pool.tile([P, P], f16, tag="attn", name="attn")
                nc.vector.tensor_scalar_mul(attn, e, rcp)
                pat = psum_tr.tile([P, tgrp * TCH], f16, tag="pt", name="pat")
                nc.tensor.transpose(pat[:, :P], attn, ids)
                attnT = sm_pool.tile([P, P], f16, tag="attnT", name="attnT")
                nc.vector.tensor_copy(out=attnT, in_=pat[:, :P])
                return attnT

            def phase5(b, attnT, p1next):
                """out = attnT.T @ (xp + yp), streamed back fp32; out chunks
                (DVE-evacuated) interleave with next batch's projection
                chunks (ACT-evacuated) so neither server throttles
                TensorE's queue."""
                xp16, yp16 = acts[b]
                for h in range(nout):
                    ot = out_pool.tile([P, out_tile], f32, tag="ot", name="ot")
                    for cc in range(cpo):
                        c0 = h * out_tile + cc * CHUNK
                        cs = slice(c0, c0 + CHUNK)
                        po = psum_out.tile([P, CHUNK], f32, tag="po", name="po")
                        nc.tensor.matmul(
                            po, attnT[:], xp16[:, cs], start=True, stop=False
                        )
                        nc.tensor.matmul(
                            po, attnT[:], yp16[:, cs], start=False, stop=True
                        )
                        nc.vector.tensor_copy(
                            out=ot[:, cc * CHUNK : (cc + 1) * CHUNK], in_=po
                        )
                        next(p1next, None)
                    hs = slice(h * out_tile, (h + 1) * out_tile)
                    # stores issue from the ACT HWDGE ring so they don't
                    # share a ring with anything hot
                    nc.scalar.dma_start(out_d[b, :, hs], ot)

            # ---- software-pipelined emission ----
            p1 = p1_chunks(0)
            for _ in p1:
                pass
            for b in range(bpc):
                ps = phase23(b)
                e, se = softmax_head(b, ps)
                p1 = p1_chunks(b + 1)
                # a few projection chunks fill TensorE while softmax runs
                next(p1, None)
                next(p1, None)
                attnT = softmax_tail(b, e, se)
                phase5(b, attnT, p1)
                for _ in p1:  # drain any leftover projection chunks
                    pass

    _split_excess_waits(nc, mybir, max_waits=1)
    return nc


_nc_cache = {}


def _get_nc():
    key = (BPC, L)
    if key not in _nc_cache:
        _nc_cache[key] = build_nc(BPC, L)
    return _nc_cache[key]


def _in_maps(x, y, Wf, bf, Wa, ba):
    wft = np.ascontiguousarray(np.asarray(Wf, dtype=np.float32).T).astype(np.float16)
    wat = np.ascontiguousarray(np.asarray(Wa, dtype=np.float32).T).astype(np.float16)
    bf2 = np.ascontiguousarray(np.asarray(bf, dtype=np.float32).reshape(P, 1))
    ba2 = np.ascontiguousarray(np.asarray(ba, dtype=np.float32).reshape(P, 1))
    maps = []
    for c in range(NCORES):
        sl = slice(c * BPC, (c + 1) * BPC)
        maps.append(
            {
                "x": np.ascontiguousarray(x[sl]),
                "y": np.ascontiguousarray(y[sl]),
                "wft": wft,
                "bf": bf2,
                "wat": wat,
                "ba": ba2,
            }
        )
    return maps


def kernel(x, y, Wf, bf, Wa, ba):
    from concourse.bass_utils import run_bass_kernel_spmd

    x = np.asarray(x, dtype=np.float32)
    y = np.asarray(y, dtype=np.float32)
    nc = _get_nc()
    res = run_bass_kernel_spmd(
        nc, _in_maps(x, y, Wf, bf, Wa, ba), core_ids=list(range(NCORES))
    )
    out = np.concatenate([r["out"] for r in res.results], axis=0)
    return np.ascontiguousarray(out.astype(np.float32))


if __name__ == "__main__":
    rng = np.random.default_rng(0)
    inputs = {
        "x": rng.standard_normal((B, P, L), dtype=np.float32),
        "y": rng.standard_normal((B, P, L), dtype=np.float32),
        "Wf": (rng.standard_normal((P, P)) / np.sqrt(P)).astype(np.float32),
        "bf": (rng.standard_normal(P) * 0.02).astype(np.float32),
        "Wa": (rng.standard_normal((P, P)) / np.sqrt(P)).astype(np.float32),
        "ba": (rng.standard_normal(P) * 0.02).astype(np.float32),
    }
    o = kernel(**inputs)
    print(o.shape, o.dtype)
